# revision 1
# baseline (speedup 1.0000x reference)
# Trainium2 Bass kernel v2 for nn_MetricLearningLoss1 (triplet loss with
# semi-hard negative mining over top-k-confidence-filtered embeddings).
#
# Strategy (8 NeuronCores, SPMD, no collectives):
#   host: top-k filter, sort by label, per-core anchor row-block + column
#         rotation (positive-label band sits at fixed compile-time offsets),
#         fp16 embeddings (pure bhi@Ehi product, rel err ~1e-4 on the loss).
#   device (per core), per 128-anchor tile, per 512-col block:
#         2 fp16 matmuls -> p[a,j] = -2*a.e_j in PSUM.
#         ACT blocks: + rank-2 matmul (ones x [sqnhi;sqnlo]) -> m = p + sqn_j;
#                     ScalarE Identity+bias(-thr-eps) -> x fp16.
#         POOL blocks: GpSimd scalar_tensor_tensor x = (p - thr-eps) + SQN16.
#         band thr: DVE eq-mask + max-reduce on ACT-block PSUM (has sqn).
#         min: DVE u16 tensor_tensor min chain over x blocks (positive fp16
#         bit patterns are order-isomorphic; negatives get the sign bit)
#         + one [128,512] u16 min reduce -> umin.
#   host: decode thr/umin -> hard_pos/hard_neg -> loss; rare rows recomputed
#         exactly on host (no semi-hard candidate / band-coverage violations).
import sys

sys.path.insert(0, "/opt/trn_rl_repo")

from contextlib import ExitStack

import numpy as np

# ---------------------------------------------------------------- constants
N_FULL, D = 32768, 256
TOPK = int(0.2 * N_FULL)  # 6553
NCORES = 8
MARGIN = np.float32(0.075)
EPS = np.float32(2e-4)

FULL_DIMS = dict(n=TOPK, npad=896, ntiles=7, W=256, blk=512)
# block -> drain engine: blocks 0..ACT_SPLIT-1 drained by ScalarE activation
# (sqn added in PSUM via a rank-2 matmul; includes band blocks 0,1), blocks
# ACT_SPLIT.. drained by DVE scalar_tensor_tensor (sqn via SQN16 in1, no PE
# cost). u16 min chain: blocks 0..MIN_SPLIT-1 on DVE, the rest on GpSimd
# (GPSIMD cannot touch PSUM, but the min chain is SBUF-only).
ACT_SPLIT = 11
MIN_SPLIT = 13
PAD_SQ = -32000.0  # per SQ2 row at padding columns; forces x << 0 there


def _nblocks(dims):
    return (dims["n"] + dims["blk"] - 1) // dims["blk"]


# ---------------------------------------------------------------- builder
def build_nc(dims, repeat=1, act_split=ACT_SPLIT, min_split=MIN_SPLIT):
    import concourse.tile as tile
    from concourse import bacc, mybir

    n, npad, ntiles, W, blk = (
        dims["n"],
        dims["npad"],
        dims["ntiles"],
        dims["W"],
        dims["blk"],
    )
    NB = _nblocks(dims)
    assert npad == ntiles * 128

    nc = bacc.Bacc(
        "TRN2", target_bir_lowering=False, debug=False, num_devices=NCORES
    )
    f16, f32, u16 = mybir.dt.float16, mybir.dt.float32, mybir.dt.uint16
    u32 = mybir.dt.uint32
    Alu = mybir.AluOpType
    Act = mybir.ActivationFunctionType
    X = mybir.AxisListType.X

    L_d = nc.dram_tensor("L", [2, 128, npad], f16, kind="ExternalInput").ap()
    R_d = nc.dram_tensor("R", [NB, 128, 2 * blk], f16, kind="ExternalInput").ap()
    EQ_d = nc.dram_tensor("EQ", [128, ntiles * W], f16, kind="ExternalInput").ap()
    f8 = mybir.dt.float8e4
    DR = mybir.MatmulPerfMode.DoubleRow
    SQ2_d = nc.dram_tensor("SQ2", [2, 2, NB * blk], f8, kind="ExternalInput").ap()
    SQB_d = nc.dram_tensor(
        "SQB", [NB - act_split, 128, blk], f16, kind="ExternalInput"
    ).ap()
    thr_d = nc.dram_tensor("thr_out", [ntiles, 128], f32, kind="ExternalOutput").ap()
    umin_d = nc.dram_tensor(
        "umin_out", [ntiles, 128], u16, kind="ExternalOutput"
    ).ap()


    with tile.TileContext(nc) as tc, ExitStack() as ctx:
        rpool = ctx.enter_context(tc.tile_pool(name="r", bufs=1))
        lpool = ctx.enter_context(tc.tile_pool(name="l", bufs=1))
        eqpool = ctx.enter_context(tc.tile_pool(name="eq", bufs=1))
        sqpool = ctx.enter_context(tc.tile_pool(name="sq", bufs=1))
        psum = ctx.enter_context(tc.tile_pool(name="ps", bufs=8, space="PSUM"))
        xpool = ctx.enter_context(tc.tile_pool(name="x", bufs=8))
        bpool = ctx.enter_context(tc.tile_pool(name="band", bufs=4))
        mpool = ctx.enter_context(tc.tile_pool(name="minacc", bufs=5))
        spool = ctx.enter_context(tc.tile_pool(name="small", bufs=6))

        # persistent inputs; emission order = DMA priority. Transfers are
        # latency-floor-bound (~500ns each), so R channels are packed into
        # one [128, 2*blk] transfer per block and EQ into one tile.
        lt = []
        for c in range(2):
            t_ = lpool.tile([128, npad], f16, tag=f"l{c}")
            lt.append(t_)
        eqall = eqpool.tile([128, ntiles * W], f16, tag="eqall")
        eqt = [eqall[:, t * W : (t + 1) * W] for t in range(ntiles)]
        # sq2 carries sqn/2 in FOUR e4m3 levels (2 partitions x 2 k-tiles,
        # residual ~8e-3; e4m3 max 240 < sqn ~ 256+, hence the /2 and the
        # 2.0 weights); still one 107ns DoubleRow matmul per block
        sq2 = sqpool.tile([2, 2, NB * blk], f8, tag="sq2")
        ones2 = sqpool.tile([2, 2, 128], f8, tag="ones2")
        nc.vector.memset(ones2[:], 2.0)

        rt = {}
        sqb = {}
        rtiles = []
        for b in range(NB):
            t_ = rpool.tile([128, 2 * blk], f16, tag=f"r{b}")
            rtiles.append(t_)
            rt[(0, b)] = t_[:, :blk]
            rt[(1, b)] = t_[:, blk:]
        # tile 0's band gates all early DVE work (in-order queue): ship the
        # minimal dep set first — L columns for tile 0 only, R0, sq2 chunk,
        # EQ[0] — then backfill
        nc.sync.dma_start(out=lt[0][:, :128], in_=L_d[0][:, :128])
        nc.sync.dma_start(out=lt[1][:, :128], in_=L_d[1][:, :128])
        nc.sync.dma_start(out=rtiles[0][:], in_=R_d[0])
        nc.sync.dma_start(out=sq2[:, :, : 2 * blk], in_=SQ2_d[:, :, : 2 * blk])
        nc.sync.dma_start(out=eqall[:, :W], in_=EQ_d[:, :W])
        nc.sync.dma_start(out=rtiles[1][:], in_=R_d[1])
        nc.sync.dma_start(out=lt[0][:, 128:], in_=L_d[0][:, 128:])
        nc.sync.dma_start(out=lt[1][:, 128:], in_=L_d[1][:, 128:])
        nc.sync.dma_start(out=eqall[:, W : 3 * W], in_=EQ_d[:, W : 3 * W])
        for b in range(2, NB):
            nc.sync.dma_start(out=rtiles[b][:], in_=R_d[b])
            if b == 3:
                nc.sync.dma_start(
                    out=sq2[:, :, 2 * blk : 6 * blk],
                    in_=SQ2_d[:, :, 2 * blk : 6 * blk],
                )
            if b == 4:
                nc.sync.dma_start(out=eqall[:, 3 * W :], in_=EQ_d[:, 3 * W :])
            if b == 6:
                hi = act_split * blk
                nc.sync.dma_start(
                    out=sq2[:, :, 6 * blk : hi], in_=SQ2_d[:, :, 6 * blk : hi]
                )
            if b >= act_split:
                t_ = sqpool.tile([128, blk], f16, tag=f"sqb{b}")
                nc.sync.dma_start(out=t_[:], in_=SQB_d[b - act_split])
                sqb[b] = t_

        state = {}

        def bw(b):
            # real width of block b: the last block is ~80% padding
            return min(blk, n - b * blk)

        def matmuls(t, b):
            st = state[t]
            w = bw(b)
            p = psum.tile([128, blk], f32, tag="pm")
            tsl = slice(128 * t, 128 * (t + 1))
            nc.tensor.matmul(
                p[:, :w], lhsT=lt[0][:, tsl], rhs=rt[(0, b)][:, :w],
                start=True, stop=False,
            )
            last = b >= act_split
            nc.tensor.matmul(
                p[:, :w], lhsT=lt[1][:, tsl], rhs=rt[(1, b)][:, :w],
                start=False, stop=last,
            )
            if not last:
                nc.tensor.matmul(
                    p[:],
                    lhsT=ones2[:],
                    rhs=sq2[:, :, blk * b : blk * (b + 1)],
                    start=False,
                    stop=True,
                    perf_mode=DR,
                )
            st["pm"][b] = p

        def xop(t, b):
            st = state[t]
            w = bw(b)
            xb = xpool.tile([128, blk], f16, tag="xb")
            if b < act_split:
                nc.scalar.activation(
                    out=xb[:, :w],
                    in_=st["pm"][b][:, :w],
                    func=Act.Identity,
                    bias=st["nthr"][:],
                    scale=1.0,
                )
            else:
                nc.vector.scalar_tensor_tensor(
                    out=xb[:, :w],
                    in0=st["pm"][b][:, :w],
                    scalar=st["nthr"][:],
                    in1=sqb[b][:, :w],
                    op0=Alu.add,
                    op1=Alu.add,
                )
            del st["pm"][b]
            st["xblks"][b] = xb

        def minop(t, b):
            st = state[t]
            w = bw(b)
            xb = st["xblks"].pop(b)
            src = xb[:, :w].bitcast(u16)
            acc = st["minacc"]
            if not st.get("min_started"):
                pend = st.get("min_pending")
                if pend is None:
                    # defer: the first two blocks' mins fuse into one op
                    assert w == blk
                    st["min_pending"] = xb
                    return
                assert w == blk
                st["min_pending"] = None
                st["min_started"] = True
                nc.vector.tensor_tensor(
                    out=acc[:], in0=pend[:].bitcast(u16), in1=src, op=Alu.min
                )
                return
            nc.vector.tensor_tensor(
                out=acc[:, :w], in0=acc[:, :w], in1=src, op=Alu.min
            )

        def band_phase(t):
            # matmuls for blocks 0,1 (they contain every band window), then
            # the masked band max -> nthr. Hoisted into tile t-1's stream so
            # the tile boundary has no band->drain serial seam.
            base = 128 * t
            parts = []
            b0 = base // blk
            off0 = base - blk * b0
            w0 = min(W, blk - off0)
            parts.append((b0, off0, 0, w0))
            if w0 < W:
                parts.append((b0 + 1, 0, w0, W - w0))
            assert parts[-1][0] <= 1 < act_split

            st = dict(pm={}, xblks={})
            state[t] = st
            minacc = mpool.tile([128, blk], u16, tag="minacc")
            st["minacc"] = minacc
            st["min_started"] = False
            matmuls(t, 0)
            matmuls(t, 1)
            accs = []
            for pi, (bb, off, eqo, bw) in enumerate(parts):
                scratch = bpool.tile([128, W], f32, tag="bsc")
                acc = spool.tile([128, 1], f32, tag=f"acc{pi}")
                nc.vector.tensor_tensor(
                    out=scratch[:, :bw],
                    in0=st["pm"][bb][:, off : off + bw],
                    in1=eqt[t][:, eqo : eqo + bw],
                    op=Alu.mult,
                )
                nc.vector.tensor_reduce(
                    out=acc[:], in_=scratch[:, :bw], axis=X, op=Alu.max
                )
                accs.append(acc)
            if len(accs) == 1:
                thr_ap = accs[0]
            else:
                thr2 = spool.tile([128, 1], f32, tag="thr2")
                nc.vector.tensor_tensor(
                    out=thr2[:], in0=accs[0][:], in1=accs[1][:], op=Alu.max
                )
                thr_ap = thr2
            nthr = spool.tile([128, 1], f32, tag="nthr")
            nc.vector.tensor_scalar(
                out=nthr[:],
                in0=thr_ap[:],
                scalar1=float(EPS),
                scalar2=-1.0,
                op0=Alu.add,
                op1=Alu.mult,
            )
            st["thr_ap"] = thr_ap
            st["nthr"] = nthr

        def finalize(t):
            st = state[t]
            # fold 512->256 at 2x-mode tt cost before the 1x-mode reduce
            acc = st["minacc"]
            nc.vector.tensor_tensor(
                out=acc[:, :256], in0=acc[:, :256], in1=acc[:, 256:], op=Alu.min
            )
            umin = spool.tile([128, 1], u16, tag="umin")
            nc.vector.tensor_reduce(
                out=umin[:], in_=acc[:, :256], axis=X, op=Alu.min
            )

            # outputs ride the SP HWDGE queue: issuing them on the
            # Activation queue punches 500ns holes into the pacing engine;
            # early tiles' outputs queue behind the input stream on SP, but
            # their buffers aren't reused until ~2 tiles later
            nc.sync.dma_start(out=thr_d[t], in_=st["thr_ap"][:, 0])
            nc.sync.dma_start(out=umin_d[t], in_=umin[:, 0])
            del state[t]

        def main_rest(t):
            st = state[t]
            for b in range(2):
                xop(t, b)
                minop(t, b)
            order = list(range(2, NB))
            if t == ntiles - 1:
                # drain the DVE-stt blocks first so the kernel tail is owned
                # by the (lighter-loaded) Activation engine
                order = list(range(act_split, NB)) + list(range(2, act_split))
            for i, b in enumerate(order):
                matmuls(t, b)
                xop(t, b)
                minop(t, b)
                if i == 4 and t + 1 < ntiles:
                    band_phase(t + 1)
            finalize(t)

        NLEAD = 2

        def sweep():
            # The input stream paces the first ~3 tiles; interleave them
            # block-major so every arriving R block feeds NLEAD tiles of
            # engine work and the DMA latency hides completely.
            for t in range(NLEAD):
                band_phase(t)
            for b in range(NB):
                for t in range(NLEAD):
                    if b >= 2:
                        matmuls(t, b)
                    xop(t, b)
                    minop(t, b)
                if b == 8:
                    band_phase(NLEAD)
            for t in range(NLEAD):
                finalize(t)
            for t in range(NLEAD, ntiles):
                main_rest(t)

        if repeat == 1:
            sweep()
        else:
            with tc.For_i(0, repeat, 1):
                sweep()

    nc.compile()
    return nc


_NC_CACHE = {}


def _get_nc(key, dims):
    if key not in _NC_CACHE:
        _NC_CACHE[key] = build_nc(dims)
    return _NC_CACHE[key]


# ---------------------------------------------------------------- host side
def host_prep(embeddings, tags, confidences, dims, act_split=ACT_SPLIT):
    n, npad, ntiles, W, blk = (
        dims["n"],
        dims["npad"],
        dims["ntiles"],
        dims["W"],
        dims["blk"],
    )
    NB = _nblocks(dims)
    ncols = NB * blk
    conf = np.asarray(confidences, dtype=np.float32)
    order = np.argsort(-conf, kind="stable")[:n]
    emb = np.asarray(embeddings, dtype=np.float32)[order]
    labs = np.asarray(tags)[order]
    perm = np.argsort(labs, kind="stable")
    emb_s = np.ascontiguousarray(emb[perm], dtype=np.float32)
    labs_s = labs[perm]
    sqn = (emb_s**2).sum(axis=1, dtype=np.float32).astype(np.float32)
    counts = np.bincount(labs_s)
    valid = (counts[labs_s] >= 2) & (counts[labs_s] < n)

    first = np.searchsorted(labs_s, labs_s, side="left")
    last = np.searchsorted(labs_s, labs_s, side="right") - 1

    # global fp16 pieces (shared across cores)
    EhiT = np.ascontiguousarray(emb_s.T.astype(np.float16))  # [256, n]
    sqnhi = sqn.astype(np.float16)
    sqnlo = (sqn - sqnhi.astype(np.float32)).astype(np.float16)

    starts = [round(k * n / NCORES) for k in range(NCORES + 1)]
    cores, in_maps = [], []
    for k in range(NCORES):
        a0, a1 = starts[k], starts[k + 1]
        cnt = a1 - a0
        s_k = (a0 - 64) % n
        labs_c = np.concatenate([labs_s[s_k:], labs_s[:s_k]])

        b = np.zeros((npad, D), np.float32)
        b[:cnt] = -2.0 * emb_s[a0:a1]
        bhiT = b.T.astype(np.float16)  # [256, npad]
        L = np.stack([bhiT[0:128], bhiT[128:256]])

        Rr = np.empty((2, 128, ncols), np.float16)
        rot = np.concatenate([EhiT[:, s_k:], EhiT[:, :s_k]], axis=1)
        Rr[0, :, :n] = rot[0:128]
        Rr[1, :, :n] = rot[128:256]
        Rr[:, :, n:] = 0
        R = np.ascontiguousarray(
            Rr.reshape(2, 128, NB, blk).transpose(2, 1, 0, 3).reshape(NB, 128, 2 * blk)
        )

        from ml_dtypes import float8_e4m3 as npf8

        q = np.concatenate([sqn[s_k:], sqn[:s_k]]) * np.float32(0.5)
        SQ2 = np.empty((2, 2, ncols), npf8)
        levels = []
        r = q
        for _ in range(4):
            s = r.astype(npf8)
            levels.append(s)
            r = r - s.astype(np.float32)
        SQ2[0, 0, :n] = levels[0]
        SQ2[0, 1, :n] = levels[1]
        SQ2[1, 0, :n] = levels[2]
        SQ2[1, 1, :n] = levels[3]
        SQ2[:, :, n:] = npf8(-240.0)  # pad: x ~ -1920 - thr - eps < 0 always
        sq32 = 2.0 * sum(
            SQ2[i, j].astype(np.float32) for i in range(2) for j in range(2)
        )
        SQB = np.ascontiguousarray(
            np.broadcast_to(
                sq32[act_split * blk :].astype(np.float16), (128, ncols - act_split * blk)
            ).reshape(128, NB - act_split, blk).transpose(1, 0, 2)
        )

        eq = np.zeros((ntiles, 128, W), np.float32)
        anchor_lab = np.full(npad, -1, np.int64)
        anchor_lab[:cnt] = labs_s[a0:a1]
        for t in range(ntiles):
            al = anchor_lab[t * 128 : (t + 1) * 128]
            w_end = min(t * 128 + W, n)
            cl = labs_c[t * 128 : w_end]
            eq[t, :, : w_end - t * 128] = (al[:, None] == cl[None, :]).astype(
                np.float32
            )
        # [128, ntiles*W] partition-major layout; fp16 (0/1 exact)
        eq = np.ascontiguousarray(
            eq.transpose(1, 0, 2).reshape(128, ntiles * W).astype(np.float16)
        )

        patch = set()
        g = np.arange(a0, a1)
        lo_loc = (first[g] - s_k) % n
        hi_loc = (last[g] - s_k) % n
        slot = np.arange(cnt)
        t_of = slot // 128
        wlo = t_of * 128
        whi = wlo + W
        bad = (lo_loc > hi_loc) | (lo_loc < wlo) | (hi_loc >= whi)
        for i in np.nonzero(bad)[0]:
            patch.add(a0 + int(i))

        cores.append(dict(a0=a0, cnt=cnt, patch=patch))
        in_maps.append({"L": L, "R": R, "EQ": eq, "SQ2": SQ2, "SQB": SQB})
    return (
        dict(emb_s=emb_s, labs_s=labs_s, sqn=sqn, valid=valid, cores=cores, n=n),
        in_maps,
    )


def host_decode(prep, outs):
    n = prep["n"]
    emb_s, labs_s, sqn, valid = (
        prep["emb_s"],
        prep["labs_s"],
        prep["sqn"],
        prep["valid"],
    )
    terms = np.zeros(n, np.float32)
    patch_rows = []
    for k, core in enumerate(prep["cores"]):
        thr_v, umin_v = outs[k]
        a0, cnt = core["a0"], core["cnt"]
        thr_v = thr_v[:cnt]
        umin_v = umin_v[:cnt]
        g = a0 + np.arange(cnt)
        vmask = valid[g]
        no_cand = umin_v >= np.uint16(0x8000)
        suspicious = (thr_v == 0.0) | no_cand
        for i in np.nonzero(vmask & suspicious)[0]:
            patch_rows.append(a0 + int(i))
        ok = vmask & ~suspicious
        if core["patch"]:
            pr = np.array(sorted(core["patch"]), np.int64) - a0
            inpr = np.zeros(cnt, bool)
            inpr[pr[(pr >= 0) & (pr < cnt)]] = True
            patch_rows.extend((a0 + np.nonzero(vmask & inpr)[0]).tolist())
            ok &= ~inpr
        idx = np.nonzero(ok)[0]
        if idx.size == 0:
            continue
        thr_ok = thr_v[idx]
        t_eps = (thr_ok + EPS).astype(np.float32)
        xstar = umin_v[idx].view(np.float16).astype(np.float32)
        mstar = (xstar + t_eps).astype(np.float32)
        gg = g[idx]
        # thr/mstar are m-values = sqn_j - 2 a.e_j ; dist^2 = m + sqn_a
        hp_sq = (thr_ok + sqn[gg]).astype(np.float32)
        hn_sq = (mstar + sqn[gg]).astype(np.float32)
        hp = np.sqrt(np.maximum(hp_sq, 0.0), dtype=np.float32)
        hn = np.sqrt(np.maximum(hn_sq, 0.0), dtype=np.float32)
        terms[gg] = np.maximum(hp - hn + MARGIN, np.float32(0.0))

    patch_rows = sorted(set(patch_rows))
    if patch_rows:
        rows = np.array(patch_rows, np.int64)
        sq_rows = (
            sqn[rows][:, None]
            + sqn[None, :]
            - 2.0 * (emb_s[rows] @ emb_s.T).astype(np.float32)
        ).astype(np.float32)
        dist = np.sqrt(np.maximum(sq_rows, 0.0), dtype=np.float32)
        for ridx, gi in enumerate(rows):
            same = labs_s == labs_s[gi]
            pos = same.copy()
            pos[gi] = False
            neg = ~same
            if not pos.any() or not neg.any():
                terms[gi] = 0.0
                continue
            drow = dist[ridx]
            hard_pos = drow[pos].max()
            neg_min = drow[neg].min()
            shn = drow[neg & (drow > hard_pos)]
            hard_neg = shn.min() if shn.size else neg_min
            terms[gi] = max(hard_pos - hard_neg + MARGIN, np.float32(0.0))

    cnt_valid = valid.sum()
    if cnt_valid > 0:
        return np.float32(terms.sum(dtype=np.float32) / max(cnt_valid, 1))
    return np.float32(0.0)


# ---------------------------------------------------------------- entry
def kernel(embeddings, tags, confidences):
    from concourse.bass_utils import run_bass_kernel_spmd

    dims = FULL_DIMS
    nc = _get_nc("full", dims)
    prep, in_maps = host_prep(embeddings, tags, confidences, dims)
    res = run_bass_kernel_spmd(nc, in_maps, list(range(NCORES)))
    outs = [
        (
            res.results[k]["thr_out"].reshape(-1).astype(np.float32),
            res.results[k]["umin_out"].reshape(-1).astype(np.uint16),
        )
        for k in range(NCORES)
    ]
    loss = host_decode(prep, outs)
    return np.array(loss, dtype=np.float32)

